# revision 2
# baseline (speedup 1.0000x reference)
"""Multi-head self-attention Trainium2 kernel (8-core SPMD).

Problem: x[4,2048,1024] -> MHSA(16 heads, d=64) -> [4,2048,1024], f32.

Sharding: core = batch*2 + head_group. Each of the 8 cores handles one
batch (of 4) and one group of 8 heads (of 16): tensor-parallel over heads
x data-parallel over batch. The final FC contraction is split over head
groups; the host sums the two partial products per batch.

Math folds (exact up to float rounding):
 - b_k drops entirely: softmax over k is invariant to a per-q shift.
 - b_v contributes P@1 * b_v = b_v per row (softmax rows sum to 1), so
   b_v @ w_fc + b_fc is a constant [1024] vector added on the host.
 - b_q is folded into the Q^T projection via the scalar engine's free
   per-partition bias on the psum-evacuating activation copy.
 - The softmax denominator Z comes free from the PV matmul: V is stored
   with a ones column appended per head, so row 64 of the PV psum is Z.
 - V is scaled x32 on the host (fp8 range) and wfc by 1/32.

Precision: projections and scores run in bf16; the PV contraction runs
in fp8-e4m3 DoubleRow (two k-tiles = 256 keys per matmul, 2x PE rate).
P^T is written by the exp directly in e4m3; V_aug is e4m3.

Engine balance (all five engines are loaded in steady state):
 - TensorE: projections/FC (128-contraction), scores (64-contraction,
   two heads row-tiled concurrently via tile_position auto-derive), PV
   (fp8 DR). PV flushes are batched two pairs at a time to halve
   64<->128 tiling-mode switches.
 - ScalarE: table exp for 9/16 k-tiles per group + Q^T bias-add psum
   evacuation + PV-psum (po->posb) evacuation + half the FC stores.
 - VectorE: Schraudolph exp for 7/16 k-tiles (uint8 bits = round(A*s+B)
   IS the e4m3 encoding of ~exp(s)), K^T/V psum evacuation, batched
   reciprocal, half the FC stores.
 - GpSimd: 1/Z partition-broadcast + the normalize multiplies + the
   wv/x2/wfc weight DMAs (keeps them off the scalar/sync queues).
 - Sync: bulk x DMAs, y stores, zrow/OT row moves.

P^T layout is (ko, h, q) so the exp engines write contiguous [128,1024]
tiles; the DR moving operand picks the pair-interleave up via a strided
3D access pattern instead.
"""

import math
import numpy as np
from contextlib import ExitStack

import concourse.bass as bass
import concourse.tile as tile
import concourse.mybir as mybir
from concourse import bacc
from concourse._compat import with_exitstack
from concourse.bass_utils import run_bass_kernel_spmd

F32 = mybir.dt.float32
F32R = mybir.dt.float32r
BF16 = mybir.dt.bfloat16
FP8 = mybir.dt.float8e4
U8 = mybir.dt.uint8
DR = mybir.MatmulPerfMode.DoubleRow

# Schraudolph exp on the DVE: uint8 bits = round(A*s + B) ARE the e4m3
# encoding of ~exp(s) (3-mantissa-bit format, bias 7 -> 8*log2(p) + 56).
# B carries a -0.5 systematic-bias tweak (DVE converts round-to-nearest).
SCH_A = 8.0 / math.log(2.0)
SCH_B = 55.5
# k-tiles whose exp runs on the DVE (rest on ScalarE table exp).
SCH_TILES = frozenset({3, 5, 7, 8, 11, 13, 15})

B, S, E = 4, 2048, 1024
H, D = 16, 64
G = 2                      # head groups (tensor parallel)
HG = H // G                # 8 heads per core
DG = HG * D                # 512 = head-group width
NCORES = B * G             # 8

DT_X = BF16                # xT / wq / wk / wv / bq / ones (proj inputs)
DT_PROJ = BF16             # wfc / OT (FC operands)
DT_ATTN = BF16             # QT / KT / V_aug / PT

EC = E // 128              # 8  e-chunks
SC = S // 512              # 4  s-chunks (q-chunks)
ST = S // 128              # 16 s-tiles (k-tiles)
DTL = DG // 128            # 4  d-tiles (head pairs)
NT = E // 128              # 8  n-tiles of output

N_WARMUP = 14              # PE clock-gate warmup matmuls (DMA lead-in)


def _np_dt(dt):
    return np.dtype(mybir.dt.np(dt))


@with_exitstack
def _emit(ctx: ExitStack, tc: tile.TileContext, io: dict):
    nc = tc.nc
    xT_d, wq_d, wk_d, wv_d, bq_d, wfc_d, yT_d = (
        io["xT"], io["wq"], io["wk"], io["wv"], io["bq"], io["wfc"], io["yT"])

    sbW = ctx.enter_context(tc.tile_pool(name="sbW", bufs=1))
    sbP = ctx.enter_context(tc.tile_pool(name="sbP", bufs=1))
    xt_pool = ctx.enter_context(tc.tile_pool(name="xt", bufs=1))
    pt_pool = ctx.enter_context(tc.tile_pool(name="pt", bufs=8))
    ev_pool = ctx.enter_context(tc.tile_pool(name="ev", bufs=6))
    nrm_pool = ctx.enter_context(tc.tile_pool(name="nrm", bufs=3))
    mm_ps = ctx.enter_context(tc.tile_pool(name="mmps", bufs=2, space="PSUM"))
    s_ps = ctx.enter_context(tc.tile_pool(name="sps", bufs=2, space="PSUM"))
    o_ps = ctx.enter_context(tc.tile_pool(name="ops", bufs=2, space="PSUM"))

    # ---- resident x^T (bf16, 4 MB, one [128, 8, 2048] tensor) + weights,
    # pre-cast to bf16 on the host. DMA queue plan (per-engine streams are
    # in-order, so issue order = need order):
    #   sync:   x0, bq, x1, x3   (x0 gates the first Q/K chain)
    #   scalar: wq, wk, wv, x2   (weights first; the x2 descriptor retires
    #                             before the first scalar compute is due)
    #   gpsimd: (wfc later, after pass A)
    xt_all = xt_pool.tile([128, EC, S], DT_X, name="xt", tag="xt")
    xTv = xT_d.rearrange("(ec p) s -> p ec s", p=128)
    wq_all = sbW.tile([128, EC, DG], DT_X, name="wq", tag="wq")
    wk_all = sbW.tile([128, EC, DG], DT_X, name="wk", tag="wk")
    wv_all = sbW.tile([128, EC, DG], DT_X, name="wv", tag="wv")
    def load_w(wt, wd, half, eng=None):
        (eng or nc.scalar).dma_start(
            wt[:, half * 4:(half + 1) * 4, :],
            wd.rearrange("(ec p) d -> p ec d",
                         p=128)[:, half * 4:(half + 1) * 4, :])

    # ---- PE warm-up: the HAM clock gate starts at half clock and only
    # ramps after ~3.4us of sustained matmul activity. Burn the DMA
    # lead-in (PE is idle anyway) on zero matmuls so the gate is warm
    # when real operands land. ----
    wu_sb = sbW.tile([128, 512], DT_X, name="wu", tag="wu")
    nc.vector.memset(wu_sb[:], 0.0)
    for i in range(N_WARMUP):
        wu_ps = o_ps.tile([65, 512], F32, name="wu_ps", tag="po")
        nc.tensor.matmul(wu_ps[:], wu_sb[:, 0:65], wu_sb[:],
                         start=True, stop=True)

    nc.sync.dma_start(xt_all[:, :, 0:512], xTv[:, :, 0:512])
    load_w(wq_all, wq_d, 0)
    load_w(wq_all, wq_d, 1)
    # bq as [128, 4]: column dt holds the 128 bias values of head-pair dt
    # (f32 so it can be the scalar-activation bias on the f32 psum)
    bq_t = sbW.tile([128, DTL], F32, name="bq", tag="bq")
    nc.sync.dma_start(bq_t[:], bq_d.rearrange("o (a p) -> (o p) a", p=128))
    load_w(wk_all, wk_d, 0)
    load_w(wk_all, wk_d, 1)
    load_w(wv_all, wv_d, 0)
    load_w(wv_all, wv_d, 1)
    nc.sync.dma_start(xt_all[:, :, 512:1024], xTv[:, :, 512:1024])
    nc.scalar.dma_start(xt_all[:, :, 1024:1536], xTv[:, :, 1024:1536])
    nc.sync.dma_start(xt_all[:, :, 1536:2048], xTv[:, :, 1536:2048])
    xt_res = [[xt_all[:, ec, sc * 512:(sc + 1) * 512] for sc in range(SC)]
              for ec in range(EC)]
    wq_t = [wq_all[:, ec, :] for ec in range(EC)]
    wk_t = [wk_all[:, ec, :] for ec in range(EC)]
    wv_t = [wv_all[:, ec, :] for ec in range(EC)]

    # ---- persistent activations (per-s-chunk tiles so attention's
    # dependencies are fine-grained and can overlap the projection) ----
    QT = [[sbP.tile([128, 512], DT_ATTN, name=f"QT{i}_{s}", tag=f"QT{i}_{s}")
           for s in range(SC)] for i in range(DTL)]
    KT = [[sbP.tile([128, 512], DT_ATTN, name=f"KT{i}_{s}", tag=f"KT{i}_{s}")
           for s in range(SC)] for i in range(DTL)]
    # V_aug in fp8 (values pre-scaled x32 on host; wfc carries 1/32), packed
    # as DoubleRow k-tile pairs: VA2[tp][:, ko, h*66+m] = V_aug for k-tile
    # 2*tp+ko, head h, column m (64 V cols + ones col at m=64; m=65 pad).
    # The ko stride (528) is a multiple of 16 as DoubleRow LDWEIGHTS needs.
    VA2 = [sbP.tile([128, 2, 8 * 66], FP8, name=f"VA2{i}", tag=f"VA2{i}")
           for i in range(ST // 2)]
    OT = [[sbP.tile([128, 512], DT_PROJ, name=f"OT{i}_{s}", tag=f"OT{i}_{s}")
           for s in range(SC)] for i in range(DTL)]

    # ones columns of V_aug (col 64 of each head's 66-col block)
    for tp in range(ST // 2):
        va4 = VA2[tp].rearrange("p ko (h c) -> p ko h c", c=66)
        nc.vector.memset(va4[:, :, :, 64:65], 1.0)

    # ---- projection pieces ----
    def emit_qk(dt_i, sc):
        """Q^T (with bias) and K^T for one head-pair tile, one s-chunk."""
        dsl = slice(dt_i * 128, (dt_i + 1) * 128)
        pq = mm_ps.tile([128, 512], F32, name="pq", tag="mm")
        for ec in range(EC):
            nc.tensor.matmul(pq[:], wq_t[ec][:, dsl], xt_res[ec][sc][:],
                             start=(ec == 0), stop=(ec == EC - 1))
        # psum evacuation with the free per-partition bias on ScalarE
        nc.scalar.add(QT[dt_i][sc][:], pq[:], bq_t[:, dt_i:dt_i + 1])
        pk = mm_ps.tile([128, 512], F32, name="pk", tag="mm")
        for ec in range(EC):
            nc.tensor.matmul(pk[:], wk_t[ec][:, dsl], xt_res[ec][sc][:],
                             start=(ec == 0), stop=(ec == EC - 1))
        nc.vector.tensor_copy(KT[dt_i][sc][:], pk[:])

    def emit_v(sc):
        for st_l in range(4):
            st = sc * 4 + st_l
            ssl = slice(st_l * 128, (st_l + 1) * 128)
            pv = mm_ps.tile([128, 512], F32, name="pv", tag="mm")
            for ec in range(EC):
                nc.tensor.matmul(pv[:], xt_res[ec][sc][:, ssl], wv_t[ec][:],
                                 start=(ec == 0), stop=(ec == EC - 1))
            va4 = VA2[st // 2].rearrange("p ko (h c) -> p ko h c", c=66)
            pv3 = pv.rearrange("p (h d) -> p h d", d=64)
            nc.vector.tensor_copy(va4[:, st % 2, :, 0:64], pv3[:])

    def emit_fc(sc):
        s0 = sc * 512
        for nt in range(NT):
            nsl = slice(nt * 128, (nt + 1) * 128)
            py = mm_ps.tile([128, 512], F32, name="py", tag="mm")
            for dt_i in range(DTL):
                nc.tensor.matmul(py[:], wfc_t[dt_i][:, nsl],
                                 OT[dt_i][sc][:],
                                 start=(dt_i == 0), stop=(dt_i == DTL - 1))
            yv = ev_pool.tile([128, 512], F32, name="yv", tag="yv")
            # alternate the psum evacuation between ScalarE and VectorE
            if nt % 2 == 0:
                nc.scalar.copy(yv[:], py[:])
            else:
                nc.vector.tensor_copy(yv[:], py[:])
            nc.sync.dma_start(yT_d[nt * 128:(nt + 1) * 128, s0:s0 + 512],
                              yv[:])

    def attn_group(hp, qc):
        """Attention for one (head-pair, q-chunk) group, split-emittable.

        Returns (pairs, finish): pairs(lo, hi) emits S/exp/PV for k-tile
        pairs [lo, hi); finish() flushes the PV pipeline and normalizes.
        Callers must emit pairs in order and only after the KT/VA2 tiles
        for that k-range have been emitted (program-order dependency).
        """
        po = [o_ps.tile([65, 512], F32, name=f"po{p}", tag="po")
              for p in range(2)]

        def emit_pv(tp, pt2):
            # DoubleRow PV: one fp8 matmul per head covers k-tile pair
            # tp (256 keys). Stationary = VA2 pair slice [128, 2, 65];
            # moving = P^T pair [128, 2, 512] (ko-strided AP).
            pv4 = pt2.rearrange("p (ko h q) -> p h ko q", ko=2, h=2)
            va4 = VA2[tp].rearrange("p ko (h c) -> p ko h c", c=66)
            for p in range(2):
                h_l = hp * 2 + p
                nc.tensor.matmul(po[p][:],
                                 va4[:, :, h_l, 0:65],
                                 pv4[:, p],
                                 start=(tp == 0),
                                 stop=(tp == ST // 2 - 1),
                                 perf_mode=DR)

        # software-pipelined k-tile pairs: S/exp of pair tp are emitted
        # ahead of the PV of pair tp-3/-4; PVs are flushed two pairs at
        # a time so the 64<->128 tiling-mode switches halve.
        pend = []  # [(tp, pt2)] awaiting PV

        def pairs(lo, hi):
            for tp in range(lo, hi):
                # pt2 free-dim layout: ko*1024 + h*512 + q -> the exp
                # engines write contiguous [128, 1024] tiles per k-tile.
                pt2 = pt_pool.tile([128, 2048], FP8, name="ptt", tag="ptt")
                pt2v = pt2.rearrange("p (ko hq) -> p ko hq", ko=2)
                pt2u = pt2.bitcast(U8).rearrange("p (ko hq) -> p ko hq",
                                                 ko=2)
                for ko in range(2):
                    kt = 2 * tp + ko
                    ps_t = s_ps.tile([128, 1024], F32, name="ps", tag="ps")
                    for p in range(2):
                        psl = slice(p * 64, (p + 1) * 64)
                        nc.tensor.matmul(ps_t[:, p * 512:(p + 1) * 512],
                                         KT[hp][kt // 4][psl, (kt % 4) * 128:
                                                         (kt % 4) * 128 + 128],
                                         QT[hp][qc][psl, :],
                                         start=True, stop=True)
                    if kt in SCH_TILES:
                        nc.vector.tensor_scalar(pt2u[:, ko], ps_t[:],
                                                SCH_A, SCH_B,
                                                mybir.AluOpType.mult,
                                                mybir.AluOpType.add)
                    else:
                        nc.scalar.activation(pt2v[:, ko], ps_t[:],
                                             mybir.ActivationFunctionType.Exp)
                pend.append((tp, pt2))
                if len(pend) >= 4:
                    emit_pv(*pend.pop(0))
                    emit_pv(*pend.pop(0))

        def finish():
            while pend:
                emit_pv(*pend.pop(0))
            _normalize()

        def _normalize():
            # Evacuate both PV psums on ScalarE (releases the po banks),
            # batch the two heads' Z rows into one reciprocal, broadcast
            # and multiply on GpSimd.
            posb = [nrm_pool.tile([65, 512], F32, name=f"posb{p}",
                                  tag=f"posb{p}") for p in range(2)]
            for p in range(2):
                nc.scalar.copy(posb[p][:], po[p][:])
            # custom DVE ops and partition_broadcast read the tensor's
            # partition 0 regardless of AP offset -> move Z via DMA first
            rz2 = nrm_pool.tile([1, 1024], F32, name="rz2", tag="rz2")
            for p in range(2):
                nc.sync.dma_start(rz2[:, p * 512:(p + 1) * 512],
                                  posb[p][64:65, :])
            rzr = nrm_pool.tile([1, 1024], F32, name="rzr", tag="rzr")
            nc.vector.reciprocal_approx_fast(rzr[:], rz2[:])
            for p in range(2):
                rzb = nrm_pool.tile([64, 512], F32, name="rzb", tag="rzb")
                nc.gpsimd.partition_broadcast(
                    rzb[:], rzr[0:1, p * 512:(p + 1) * 512])
                if p == 0:
                    nc.gpsimd.tensor_mul(OT[hp][qc][0:64, :],
                                         posb[0][0:64, :], rzb[:])
                else:
                    tmp = nrm_pool.tile([64, 512], DT_PROJ,
                                        name="otmp", tag="otmp")
                    nc.gpsimd.tensor_mul(tmp[:], posb[1][0:64, :], rzb[:])
                    # engines cannot shift partitions; DMA moves rows
                    # 0:64 into OT rows 64:128.
                    nc.sync.dma_start(OT[hp][qc][64:128, :], tmp[:])

        return pairs, finish

    def emit_attn(hp, qc):
        pairs, finish = attn_group(hp, qc)
        pairs(0, ST // 2)
        finish()

    # ---- pass A: V (all heads) + Q/K for head-pair 0, with head-pair
    # 0's first attention group interleaved so the exp engines start as
    # soon as the first chunk's Q/K land (S/exp of pairs [lo,hi) only
    # needs KT; the PV of pair tp is emitted a few pairs later, by which
    # point the V chains covering it are in program order). ----
    g0_pairs, g0_finish = attn_group(0, 0)
    for sc in range(SC):
        emit_qk(0, sc)
        emit_v(sc)
        g0_pairs(2 * sc, 2 * sc + 2)
    g0_finish()
    for qc in range(1, SC):
        emit_attn(0, qc)

    # wfc loads deferred past pass A on the (otherwise idle) gpsimd
    # queue: first FC use is in the hp3 window.
    wfc_t = []
    for dt_i in range(DTL):
        t = sbW.tile([128, E], DT_PROJ, name=f"wfc{dt_i}", tag=f"wfc{dt_i}")
        nc.gpsimd.dma_start(t[:], wfc_d[dt_i * 128:(dt_i + 1) * 128, :])
        wfc_t.append(t)

    # ---- attention interleaved with deferred projections ----
    # Attention for head-pair hp runs while the projection for head-pair
    # hp+1 (emitted just after, lower priority) fills PE gaps. In the
    # last head-pair, the FC of the previous q-chunk is emitted into the
    # middle of the group so it overlaps the exp-bound stretch instead of
    # piling up after the last group.
    for hp in range(1, DTL):
        for sc in range(SC):
            emit_qk(hp, sc)
        for qc in range(SC):
            pairs, finishg = attn_group(hp, qc)
            pairs(0, 4)
            if hp == DTL - 1 and qc >= 1:
                emit_fc(qc - 1)
            pairs(4, ST // 2)
            finishg()
    emit_fc(SC - 1)

_CACHE = {}


def _build():
    if "nc" in _CACHE:
        return _CACHE["nc"]
    nc = bacc.Bacc("TRN2", target_bir_lowering=False, debug=False)
    io = {
        "xT": nc.dram_tensor("xT", [E, S], BF16, kind="ExternalInput").ap(),
        "wq": nc.dram_tensor("wq", [E, DG], BF16, kind="ExternalInput").ap(),
        "wk": nc.dram_tensor("wk", [E, DG], BF16, kind="ExternalInput").ap(),
        "wv": nc.dram_tensor("wv", [E, DG], BF16, kind="ExternalInput").ap(),
        "bq": nc.dram_tensor("bq", [1, DG], F32, kind="ExternalInput").ap(),
        "wfc": nc.dram_tensor("wfc", [DG, E], BF16,
                              kind="ExternalInput").ap(),
        "yT": nc.dram_tensor("yT", [E, S], F32, kind="ExternalOutput").ap(),
    }
    with tile.TileContext(nc) as tc:
        _emit(tc, io)
    nc.compile()
    _CACHE["nc"] = nc
    return nc


def make_in_maps(x, w_qkv, b_qkv, w_fc):
    """Host-side sharding: returns per-core input dicts (core = b*G + g)."""
    import ml_dtypes
    x = np.asarray(x, dtype=np.float32)
    w_qkv = np.asarray(w_qkv, dtype=np.float32)
    b_qkv = np.asarray(b_qkv, dtype=np.float32)
    w_fc = np.asarray(w_fc, dtype=np.float32)
    npdt = ml_dtypes.bfloat16
    in_maps = []
    for b in range(B):
        xTb = np.ascontiguousarray(x[b].T).astype(npdt)
        for g in range(G):
            gs = slice(g * DG, (g + 1) * DG)
            in_maps.append({
                "xT": xTb,
                "wq": np.ascontiguousarray(
                    w_qkv[:, 0 * E:1 * E][:, gs] * (1.0 / np.sqrt(D))
                ).astype(npdt),
                "wk": np.ascontiguousarray(w_qkv[:, 1 * E:2 * E][:, gs]).astype(npdt),
                # V path scaled x32 so the fp8(e4m3) V_aug values sit in the
                # normal range; wfc carries the 1/32 back out.
                "wv": np.ascontiguousarray(
                    w_qkv[:, 2 * E:3 * E][:, gs] * 32.0).astype(npdt),
                "bq": np.ascontiguousarray(
                    b_qkv[0 * E:1 * E][gs][None, :] * (1.0 / np.sqrt(D))
                ).astype(np.float32),
                "wfc": np.ascontiguousarray(
                    w_fc[gs, :] * (1.0 / 32.0)).astype(npdt),
            })
    return in_maps


def gather(results, b_qkv, w_fc, b_fc):
    """Host-side unshard: sum group partials, transpose, add const bias."""
    b_qkv = np.asarray(b_qkv, dtype=np.float32)
    w_fc = np.asarray(w_fc, dtype=np.float32)
    b_fc = np.asarray(b_fc, dtype=np.float32)
    cbias = (b_qkv[2 * E:3 * E].astype(np.float64) @ w_fc.astype(np.float64)
             + b_fc.astype(np.float64)).astype(np.float32)
    y = np.empty((B, S, E), np.float32)
    for b in range(B):
        yT = results[b * G]["yT"] + results[b * G + 1]["yT"]
        y[b] = yT.T + cbias[None, :]
    return y


def kernel(x, w_qkv, b_qkv, w_fc, b_fc, _trace=False, _tmpdir=None):
    nc = _build()
    in_maps = make_in_maps(x, w_qkv, b_qkv, w_fc)
    res = run_bass_kernel_spmd(nc, in_maps, list(range(NCORES)),
                               trace=_trace, tmpdir=_tmpdir)
    y = gather(res.results, b_qkv, w_fc, b_fc)
    kernel.last_exec_time_ns = res.exec_time_ns
    kernel.last_res = res
    return y


# revision 9
# speedup vs baseline: 1.2161x; 1.2161x over previous
"""Multi-head self-attention Trainium2 kernel (8-core SPMD).

Problem: x[4,2048,1024] -> MHSA(16 heads, d=64) -> [4,2048,1024], f32.

Sharding: core = batch*2 + head_group. Each of the 8 cores handles one
batch (of 4) and one group of 8 heads (of 16): tensor-parallel over heads
x data-parallel over batch. The final FC contraction is split over head
groups; the host sums the two partial products per batch.

Math folds (exact up to float rounding):
 - b_k drops entirely: softmax over k is invariant to a per-q shift.
 - b_v contributes P@1 * b_v = b_v per row (softmax rows sum to 1), so
   b_v @ w_fc + b_fc is a constant [1024] vector added on the host.
 - b_q is folded into the Q^T projection via the scalar engine's free
   per-partition bias on the psum-evacuating activation copy.
 - The softmax denominator Z comes free from the PV matmul: V is stored
   with a ones column appended per head, so row 64 of the PV psum is Z.
 - V is scaled x32 on the host (fp8 range) and wfc by 1/32.

Precision: projections and scores run in bf16; the PV contraction runs
in fp8-e4m3 DoubleRow (two k-tiles = 256 keys per matmul, 2x PE rate).
P^T is written by the exp directly in e4m3; V_aug is e4m3.

Engine balance (all five engines are loaded in steady state):
 - TensorE: projections/FC (128-contraction), scores (64-contraction,
   two heads row-tiled concurrently via tile_position auto-derive), PV
   (fp8 DR). PV flushes are batched two pairs at a time to halve
   64<->128 tiling-mode switches.
 - ScalarE: table exp for 9/16 k-tiles per group + Q^T bias-add psum
   evacuation + PV-psum (po->posb) evacuation + half the FC stores.
 - VectorE: Schraudolph exp for 7/16 k-tiles (uint8 bits = round(A*s+B)
   IS the e4m3 encoding of ~exp(s)), K^T/V psum evacuation, batched
   reciprocal, half the FC stores.
 - GpSimd: 1/Z partition-broadcast + the normalize multiplies + the
   wv/x2/wfc weight DMAs (keeps them off the scalar/sync queues).
 - Sync: bulk x DMAs, y stores, zrow/OT row moves.

P^T layout is (ko, h, q) so the exp engines write contiguous [128,1024]
tiles; the DR moving operand picks the pair-interleave up via a strided
3D access pattern instead.
"""

import math
import numpy as np
from contextlib import ExitStack

import concourse.bass as bass
import concourse.tile as tile
import concourse.mybir as mybir
from concourse import bacc
from concourse._compat import with_exitstack
from concourse.bass_utils import run_bass_kernel_spmd

F32 = mybir.dt.float32
F32R = mybir.dt.float32r
BF16 = mybir.dt.bfloat16
FP8 = mybir.dt.float8e4
U8 = mybir.dt.uint8
DR = mybir.MatmulPerfMode.DoubleRow

# Schraudolph exp on the DVE: uint8 bits = round(A*s + B) ARE the e4m3
# encoding of ~exp(s) (3-mantissa-bit format, bias 7 -> 8*log2(p) + 56).
# B carries a -0.5 systematic-bias tweak (DVE converts round-to-nearest).
SCH_A = 8.0 / math.log(2.0)
SCH_B = 55.5
# k-tiles whose exp runs on the DVE (rest on ScalarE table exp).
SCH_TILES = frozenset({1, 4, 7, 10, 13})

B, S, E = 4, 2048, 1024
H, D = 16, 64
G = 2                      # head groups (tensor parallel)
HG = H // G                # 8 heads per core
DG = HG * D                # 512 = head-group width
NCORES = B * G             # 8

DT_X = BF16                # xT / wq / wk / wv / bq / ones (proj inputs)
DT_PROJ = BF16             # wfc / OT (FC operands)
DT_ATTN = BF16             # QT / KT / V_aug / PT

EC = E // 128              # 8  e-chunks
SC = S // 512              # 4  s-chunks (q-chunks)
ST = S // 128              # 16 s-tiles (k-tiles)
DTL = DG // 128            # 4  d-tiles (head pairs)
NT = E // 128              # 8  n-tiles of output

N_WARMUP = 18              # PE clock-gate warmup matmuls (DMA lead-in)


def _np_dt(dt):
    return np.dtype(mybir.dt.np(dt))


@with_exitstack
def _emit(ctx: ExitStack, tc: tile.TileContext, io: dict):
    nc = tc.nc
    xT_d, wq_d, wk_d, wv_d, bq_d, wfc_d, yT_d = (
        io["xT"], io["wq"], io["wk"], io["wv"], io["bq"], io["wfc"], io["yT"])

    sbW = ctx.enter_context(tc.tile_pool(name="sbW", bufs=1))
    sbP = ctx.enter_context(tc.tile_pool(name="sbP", bufs=1))
    xt_pool = ctx.enter_context(tc.tile_pool(name="xt", bufs=1))
    pt_pool = ctx.enter_context(tc.tile_pool(name="pt", bufs=8))
    ev_pool = ctx.enter_context(tc.tile_pool(name="ev", bufs=6))
    nrm_pool = ctx.enter_context(tc.tile_pool(name="nrm", bufs=3))
    mm_ps = ctx.enter_context(tc.tile_pool(name="mmps", bufs=2, space="PSUM"))
    s_ps = ctx.enter_context(tc.tile_pool(name="sps", bufs=2, space="PSUM"))
    o_ps = ctx.enter_context(tc.tile_pool(name="ops", bufs=2, space="PSUM"))

    # ---- resident x^T (bf16, 4 MB, one [128, 8, 2048] tensor) + weights,
    # pre-cast to bf16 on the host. DMA queue plan (per-engine streams are
    # in-order, so issue order = need order):
    #   sync:   x0, bq, x1, x3   (x0 gates the first Q/K chain)
    #   scalar: wq, wk, wv, x2   (weights first; the x2 descriptor retires
    #                             before the first scalar compute is due)
    #   gpsimd: (wfc later, after pass A)
    xt_all = xt_pool.tile([128, EC, S], DT_X, name="xt", tag="xt")
    xTv = xT_d.rearrange("(ec p) s -> p ec s", p=128)
    wq_all = sbW.tile([128, EC, DG], DT_X, name="wq", tag="wq")
    wk_all = sbW.tile([128, EC, DG], DT_X, name="wk", tag="wk")
    wv_all = sbW.tile([128, EC, DG], DT_X, name="wv", tag="wv")
    def load_w(wt, wd, half, eng=None):
        (eng or nc.scalar).dma_start(
            wt[:, half * 4:(half + 1) * 4, :],
            wd.rearrange("(ec p) d -> p ec d",
                         p=128)[:, half * 4:(half + 1) * 4, :])

    # ---- PE warm-up: the HAM clock gate starts at half clock and only
    # ramps after ~3.4us of sustained matmul activity. Burn the DMA
    # lead-in (PE is idle anyway) on zero matmuls so the gate is warm
    # when real operands land. ----
    wu_sb = sbW.tile([128, 512], DT_X, name="wu", tag="wu")
    nc.vector.memset(wu_sb[:], 0.0)
    for i in range(N_WARMUP):
        wu_ps = o_ps.tile([65, 512], F32, name="wu_ps", tag="po")
        nc.tensor.matmul(wu_ps[:], wu_sb[:, 0:65], wu_sb[:],
                         start=True, stop=True)

    nc.sync.dma_start(xt_all[:, :, 0:512], xTv[:, :, 0:512])
    load_w(wq_all, wq_d, 0)
    load_w(wq_all, wq_d, 1)
    # bq as [128, 4]: column dt holds the 128 bias values of head-pair dt
    # (f32 so it can be the scalar-activation bias on the f32 psum)
    bq_t = sbW.tile([128, DTL], F32, name="bq", tag="bq")
    nc.sync.dma_start(bq_t[:], bq_d.rearrange("o (a p) -> (o p) a", p=128))
    load_w(wk_all, wk_d, 0)
    load_w(wk_all, wk_d, 1)
    load_w(wv_all, wv_d, 0)
    load_w(wv_all, wv_d, 1)
    for sc in range(1, SC):
        nc.sync.dma_start(xt_all[:, :, sc * 512:(sc + 1) * 512],
                          xTv[:, :, sc * 512:(sc + 1) * 512])
    xt_res = [[xt_all[:, ec, sc * 512:(sc + 1) * 512] for sc in range(SC)]
              for ec in range(EC)]
    wq_t = [wq_all[:, ec, :] for ec in range(EC)]
    wk_t = [wk_all[:, ec, :] for ec in range(EC)]
    wv_t = [wv_all[:, ec, :] for ec in range(EC)]

    # ---- persistent activations (per-s-chunk tiles so attention's
    # dependencies are fine-grained and can overlap the projection) ----
    QT = [[sbP.tile([128, 512], DT_ATTN, name=f"QT{i}_{s}", tag=f"QT{i}_{s}")
           for s in range(SC)] for i in range(DTL)]
    KT = [[sbP.tile([128, 512], DT_ATTN, name=f"KT{i}_{s}", tag=f"KT{i}_{s}")
           for s in range(SC)] for i in range(DTL)]
    # V_aug in fp8 (values pre-scaled x32 on host; wfc carries 1/32), packed
    # as DoubleRow k-tile pairs: VA2[tp][:, ko, h*66+m] = V_aug for k-tile
    # 2*tp+ko, head h, column m (64 V cols + ones col at m=64; m=65 pad).
    # The ko stride (528) is a multiple of 16 as DoubleRow LDWEIGHTS needs.
    VA2 = [sbP.tile([128, 2, 8 * 66], FP8, name=f"VA2{i}", tag=f"VA2{i}")
           for i in range(ST // 2)]
    OT = [[sbP.tile([128, 512], DT_PROJ, name=f"OT{i}_{s}", tag=f"OT{i}_{s}")
           for s in range(SC)] for i in range(DTL)]

    # ones columns of V_aug (col 64 of each head's 66-col block)
    for tp in range(ST // 2):
        va4 = VA2[tp].rearrange("p ko (h c) -> p ko h c", c=66)
        nc.vector.memset(va4[:, :, :, 64:65], 1.0)

    # ---- projection pieces ----
    def emit_qk(dt_i, sc):
        """Q^T (with bias) and K^T for one head-pair tile, one s-chunk."""
        dsl = slice(dt_i * 128, (dt_i + 1) * 128)
        pq = mm_ps.tile([128, 512], F32, name="pq", tag="mm")
        for ec in range(EC):
            nc.tensor.matmul(pq[:], wq_t[ec][:, dsl], xt_res[ec][sc][:],
                             start=(ec == 0), stop=(ec == EC - 1))
        nc.vector.tensor_scalar_add(QT[dt_i][sc][:], pq[:],
                                    bq_t[:, dt_i:dt_i + 1])
        pk = mm_ps.tile([128, 512], F32, name="pk", tag="mm")
        for ec in range(EC):
            nc.tensor.matmul(pk[:], wk_t[ec][:, dsl], xt_res[ec][sc][:],
                             start=(ec == 0), stop=(ec == EC - 1))
        nc.vector.tensor_copy(KT[dt_i][sc][:], pk[:])

    def emit_v(sc):
        for st_l in range(4):
            st = sc * 4 + st_l
            ssl = slice(st_l * 128, (st_l + 1) * 128)
            pv = mm_ps.tile([128, 512], F32, name="pv", tag="mm")
            for ec in range(EC):
                nc.tensor.matmul(pv[:], xt_res[ec][sc][:, ssl], wv_t[ec][:],
                                 start=(ec == 0), stop=(ec == EC - 1))
            va4 = VA2[st // 2].rearrange("p ko (h c) -> p ko h c", c=66)
            pv3 = pv.rearrange("p (h d) -> p h d", d=64)
            nc.vector.tensor_copy(va4[:, st % 2, :, 0:64], pv3[:])

    def emit_fc(sc):
        s0 = sc * 512
        for nt in range(NT):
            nsl = slice(nt * 128, (nt + 1) * 128)
            py = mm_ps.tile([128, 512], F32, name="py", tag="mm")
            for dt_i in range(DTL):
                nc.tensor.matmul(py[:], wfc_t[dt_i][:, nsl],
                                 OT[dt_i][sc][:],
                                 start=(dt_i == 0), stop=(dt_i == DTL - 1))
            yv = ev_pool.tile([128, 512], F32, name="yv", tag="yv")
            nc.vector.tensor_copy(yv[:], py[:])
            nc.sync.dma_start(yT_d[nt * 128:(nt + 1) * 128, s0:s0 + 512],
                              yv[:])

    def attn_group(hp, qc):
        """Attention for one (head-pair, q-chunk) group, split-emittable.

        Returns (pairs, finish): pairs(lo, hi) emits S/exp/PV for k-tile
        pairs [lo, hi); finish() flushes the PV pipeline and normalizes.
        Callers must emit pairs in order and only after the KT/VA2 tiles
        for that k-range have been emitted (program-order dependency).
        """
        po = [o_ps.tile([65, 512], F32, name=f"po{p}", tag="po")
              for p in range(2)]

        def emit_pv(tp, pt2):
            # DoubleRow PV: one fp8 matmul per head covers k-tile pair
            # tp (256 keys). Stationary = VA2 pair slice [128, 2, 65];
            # moving = P^T pair [128, 2, 512] (ko-strided AP).
            pv4 = pt2.rearrange("p (ko h q) -> p h ko q", ko=2, h=2)
            va4 = VA2[tp].rearrange("p ko (h c) -> p ko h c", c=66)
            for p in range(2):
                h_l = hp * 2 + p
                nc.tensor.matmul(po[p][:],
                                 va4[:, :, h_l, 0:65],
                                 pv4[:, p],
                                 start=(tp == 0),
                                 stop=(tp == ST // 2 - 1),
                                 perf_mode=DR)

        # software-pipelined k-tile pairs: S/exp of pair tp are emitted
        # ahead of the PV of pair tp-3/-4; PVs are flushed two pairs at
        # a time so the 64<->128 tiling-mode switches halve.
        pend = []  # [(tp, pt2)] awaiting PV

        def pairs(lo, hi):
            for tp in range(lo, hi):
                # pt2 free-dim layout: ko*1024 + h*512 + q -> the exp
                # engines write contiguous [128, 1024] tiles per k-tile.
                pt2 = pt_pool.tile([128, 2048], FP8, name="ptt", tag="ptt")
                pt2v = pt2.rearrange("p (ko hq) -> p ko hq", ko=2)
                pt2u = pt2.bitcast(U8).rearrange("p (ko hq) -> p ko hq",
                                                 ko=2)
                for ko in range(2):
                    kt = 2 * tp + ko
                    ps_t = s_ps.tile([128, 1024], F32, name="ps", tag="ps")
                    for p in range(2):
                        psl = slice(p * 64, (p + 1) * 64)
                        nc.tensor.matmul(ps_t[:, p * 512:(p + 1) * 512],
                                         KT[hp][kt // 4][psl, (kt % 4) * 128:
                                                         (kt % 4) * 128 + 128],
                                         QT[hp][qc][psl, :],
                                         start=True, stop=True)
                    if kt in SCH_TILES:
                        nc.vector.tensor_scalar(pt2u[:, ko], ps_t[:],
                                                SCH_A, SCH_B,
                                                mybir.AluOpType.mult,
                                                mybir.AluOpType.add)
                    else:
                        nc.scalar.activation(pt2v[:, ko], ps_t[:],
                                             mybir.ActivationFunctionType.Exp)
                pend.append((tp, pt2))
                if len(pend) >= 4:
                    emit_pv(*pend.pop(0))
                    emit_pv(*pend.pop(0))

        def finish():
            while pend:
                emit_pv(*pend.pop(0))
            _normalize()

        def _normalize():
            # Evacuate both PV psums on ScalarE (releases the po banks),
            # batch the two heads' Z rows into one reciprocal, broadcast
            # and multiply on GpSimd.
            posb = [nrm_pool.tile([65, 512], F32, name=f"posb{p}",
                                  tag=f"posb{p}") for p in range(2)]
            for p in range(2):
                nc.vector.tensor_copy(posb[p][:], po[p][:])
            # custom DVE ops and partition_broadcast read the tensor's
            # partition 0 regardless of AP offset -> move Z via DMA first
            rz2 = nrm_pool.tile([1, 1024], F32, name="rz2", tag="rz2")
            for p in range(2):
                nc.sync.dma_start(rz2[:, p * 512:(p + 1) * 512],
                                  posb[p][64:65, :])
            rzr = nrm_pool.tile([1, 1024], F32, name="rzr", tag="rzr")
            nc.vector.reciprocal_approx_fast(rzr[:], rz2[:])
            for p in range(2):
                rzb = nrm_pool.tile([64, 512], F32, name="rzb", tag="rzb")
                nc.gpsimd.partition_broadcast(
                    rzb[:], rzr[0:1, p * 512:(p + 1) * 512])
                if p == 0:
                    nc.gpsimd.tensor_mul(OT[hp][qc][0:64, :],
                                         posb[0][0:64, :], rzb[:])
                else:
                    tmp = nrm_pool.tile([64, 512], DT_PROJ,
                                        name="otmp", tag="otmp")
                    nc.gpsimd.tensor_mul(tmp[:], posb[1][0:64, :], rzb[:])
                    # engines cannot shift partitions; DMA moves rows
                    # 0:64 into OT rows 64:128.
                    nc.sync.dma_start(OT[hp][qc][64:128, :], tmp[:])

        return pairs, finish

    def emit_attn(hp, qc):
        pairs, finish = attn_group(hp, qc)
        pairs(0, ST // 2)
        finish()

    # ---- pass A: V (all heads) + Q/K for head-pair 0, with head-pair
    # 0's first attention group interleaved so the exp engines start as
    # soon as the first chunk's Q/K land (S/exp of pairs [lo,hi) only
    # needs KT; the PV of pair tp is emitted a few pairs later, by which
    # point the V chains covering it are in program order). ----
    g0_pairs, g0_finish = attn_group(0, 0)
    for sc in range(SC):
        emit_qk(0, sc)
        emit_v(sc)
        g0_pairs(2 * sc, 2 * sc + 2)
    g0_finish()
    for qc in range(1, SC):
        emit_attn(0, qc)

    # wfc loads deferred past pass A: first FC use is in the hp3 window,
    # and these 2MB would otherwise crowd the DMA-paced startup. On the
    # sync queue they naturally sit behind the x chunks + g0's moves.
    wfc_t = []
    for dt_i in range(DTL):
        t = sbW.tile([128, E], DT_PROJ, name=f"wfc{dt_i}", tag=f"wfc{dt_i}")
        nc.sync.dma_start(t[:], wfc_d[dt_i * 128:(dt_i + 1) * 128, :])
        wfc_t.append(t)

    # ---- attention interleaved with deferred projections ----
    # Attention for head-pair hp runs while the projection for head-pair
    # hp+1 (emitted just after, lower priority) fills PE gaps. In the
    # last head-pair, the FC of the previous q-chunk is emitted into the
    # middle of the group so it overlaps the exp-bound stretch instead of
    # piling up after the last group.
    for hp in range(1, DTL):
        for sc in range(SC):
            emit_qk(hp, sc)
        for qc in range(SC):
            pairs, finishg = attn_group(hp, qc)
            pairs(0, 4)
            if hp == DTL - 1 and qc >= 1:
                emit_fc(qc - 1)
            pairs(4, ST // 2)
            finishg()
    emit_fc(SC - 1)

_CACHE = {}


def _build():
    if "nc" in _CACHE:
        return _CACHE["nc"]
    nc = bacc.Bacc("TRN2", target_bir_lowering=False, debug=False)
    io = {
        "xT": nc.dram_tensor("xT", [E, S], BF16, kind="ExternalInput").ap(),
        "wq": nc.dram_tensor("wq", [E, DG], BF16, kind="ExternalInput").ap(),
        "wk": nc.dram_tensor("wk", [E, DG], BF16, kind="ExternalInput").ap(),
        "wv": nc.dram_tensor("wv", [E, DG], BF16, kind="ExternalInput").ap(),
        "bq": nc.dram_tensor("bq", [1, DG], F32, kind="ExternalInput").ap(),
        "wfc": nc.dram_tensor("wfc", [DG, E], BF16,
                              kind="ExternalInput").ap(),
        "yT": nc.dram_tensor("yT", [E, S], F32, kind="ExternalOutput").ap(),
    }
    with tile.TileContext(nc) as tc:
        _emit(tc, io)
    nc.compile()
    _CACHE["nc"] = nc
    return nc


def make_in_maps(x, w_qkv, b_qkv, w_fc):
    """Host-side sharding: returns per-core input dicts (core = b*G + g)."""
    import ml_dtypes
    x = np.asarray(x, dtype=np.float32)
    w_qkv = np.asarray(w_qkv, dtype=np.float32)
    b_qkv = np.asarray(b_qkv, dtype=np.float32)
    w_fc = np.asarray(w_fc, dtype=np.float32)
    npdt = ml_dtypes.bfloat16
    in_maps = []
    for b in range(B):
        xTb = np.ascontiguousarray(x[b].T).astype(npdt)
        for g in range(G):
            gs = slice(g * DG, (g + 1) * DG)
            in_maps.append({
                "xT": xTb,
                "wq": np.ascontiguousarray(
                    w_qkv[:, 0 * E:1 * E][:, gs] * (1.0 / np.sqrt(D))
                ).astype(npdt),
                "wk": np.ascontiguousarray(w_qkv[:, 1 * E:2 * E][:, gs]).astype(npdt),
                # V path scaled x32 so the fp8(e4m3) V_aug values sit in the
                # normal range; wfc carries the 1/32 back out.
                "wv": np.ascontiguousarray(
                    w_qkv[:, 2 * E:3 * E][:, gs] * 32.0).astype(npdt),
                "bq": np.ascontiguousarray(
                    b_qkv[0 * E:1 * E][gs][None, :] * (1.0 / np.sqrt(D))
                ).astype(np.float32),
                "wfc": np.ascontiguousarray(
                    w_fc[gs, :] * (1.0 / 32.0)).astype(npdt),
            })
    return in_maps


def gather(results, b_qkv, w_fc, b_fc):
    """Host-side unshard: sum group partials, transpose, add const bias."""
    b_qkv = np.asarray(b_qkv, dtype=np.float32)
    w_fc = np.asarray(w_fc, dtype=np.float32)
    b_fc = np.asarray(b_fc, dtype=np.float32)
    cbias = (b_qkv[2 * E:3 * E].astype(np.float64) @ w_fc.astype(np.float64)
             + b_fc.astype(np.float64)).astype(np.float32)
    y = np.empty((B, S, E), np.float32)
    for b in range(B):
        yT = results[b * G]["yT"] + results[b * G + 1]["yT"]
        y[b] = yT.T + cbias[None, :]
    return y


def kernel(x, w_qkv, b_qkv, w_fc, b_fc, _trace=False, _tmpdir=None):
    nc = _build()
    in_maps = make_in_maps(x, w_qkv, b_qkv, w_fc)
    res = run_bass_kernel_spmd(nc, in_maps, list(range(NCORES)),
                               trace=_trace, tmpdir=_tmpdir)
    y = gather(res.results, b_qkv, w_fc, b_fc)
    kernel.last_exec_time_ns = res.exec_time_ns
    kernel.last_res = res
    return y


# revision 18
# speedup vs baseline: 1.2350x; 1.0155x over previous
"""Multi-head self-attention Trainium2 kernel (8-core SPMD).

Problem: x[4,2048,1024] -> MHSA(16 heads, d=64) -> [4,2048,1024], f32.

Sharding: core = batch*2 + head_group. Each of the 8 cores handles one
batch (of 4) and one group of 8 heads (of 16): tensor-parallel over heads
x data-parallel over batch. The final FC contraction is split over head
groups; the host sums the two partial products per batch.

Math folds (exact up to float rounding):
 - b_k drops entirely: softmax over k is invariant to a per-q shift.
 - b_v contributes P@1 * b_v = b_v per row (softmax rows sum to 1), so
   b_v @ w_fc + b_fc is a constant [1024] vector added on the host.
 - b_q is folded into the Q^T projection via the scalar engine's free
   per-partition bias on the psum-evacuating activation copy.
 - The softmax denominator Z comes free from the PV matmul: V is stored
   with a ones column appended per head, so row 64 of the PV psum is Z.
 - V is scaled x32 on the host (fp8 range) and wfc by 1/32.

Precision: projections and scores run in bf16; the PV contraction runs
in fp8-e4m3 DoubleRow (two k-tiles = 256 keys per matmul, 2x PE rate).
P^T is written by the exp directly in e4m3; V_aug is e4m3.

Engine balance (all five engines are loaded in steady state):
 - TensorE: projections/FC (128-contraction), scores (64-contraction,
   two heads row-tiled concurrently via tile_position auto-derive), PV
   (fp8 DR). PV flushes are batched two pairs at a time to halve
   64<->128 tiling-mode switches.
 - ScalarE: table exp for 9/16 k-tiles per group + Q^T bias-add psum
   evacuation + PV-psum (po->posb) evacuation + half the FC stores.
 - VectorE: Schraudolph exp for 7/16 k-tiles (uint8 bits = round(A*s+B)
   IS the e4m3 encoding of ~exp(s)), K^T/V psum evacuation, batched
   reciprocal, half the FC stores.
 - GpSimd: 1/Z partition-broadcast + the normalize multiplies + the
   wv/x2/wfc weight DMAs (keeps them off the scalar/sync queues).
 - Sync: bulk x DMAs, y stores, zrow/OT row moves.

P^T layout is (ko, h, q) so the exp engines write contiguous [128,1024]
tiles; the DR moving operand picks the pair-interleave up via a strided
3D access pattern instead.
"""

import math
import numpy as np
from contextlib import ExitStack

import concourse.bass as bass
import concourse.tile as tile
import concourse.mybir as mybir
from concourse import bacc
from concourse._compat import with_exitstack
from concourse.bass_utils import run_bass_kernel_spmd

F32 = mybir.dt.float32
F32R = mybir.dt.float32r
BF16 = mybir.dt.bfloat16
FP8 = mybir.dt.float8e4
U8 = mybir.dt.uint8
DR = mybir.MatmulPerfMode.DoubleRow

# Schraudolph exp on the DVE: uint8 bits = round(A*s + B) ARE the e4m3
# encoding of ~exp(s) (3-mantissa-bit format, bias 7 -> 8*log2(p) + 56).
# B carries a -0.5 systematic-bias tweak (DVE converts round-to-nearest).
SCH_A = 8.0 / math.log(2.0)
SCH_B = 55.5
# k-tile PAIRS whose exp runs on the DVE (rest on ScalarE table exp).
# Pair-granular so each P^T tile is written by exactly one engine and the
# two engines' tile rings never couple through write-after-write ordering.
SCH_PAIRS = frozenset({1, 4, 6})

B, S, E = 4, 2048, 1024
H, D = 16, 64
G = 2                      # head groups (tensor parallel)
HG = H // G                # 8 heads per core
DG = HG * D                # 512 = head-group width
NCORES = B * G             # 8

DT_X = BF16                # xT / wq / wk / wv / bq / ones (proj inputs)
DT_PROJ = BF16             # wfc / OT (FC operands)
DT_ATTN = BF16             # QT / KT / V_aug / PT

EC = E // 128              # 8  e-chunks
SC = S // 512              # 4  s-chunks (q-chunks)
ST = S // 128              # 16 s-tiles (k-tiles)
DTL = DG // 128            # 4  d-tiles (head pairs)
NT = E // 128              # 8  n-tiles of output

N_WARMUP = 18              # PE clock-gate warmup matmuls (DMA lead-in)


def _np_dt(dt):
    return np.dtype(mybir.dt.np(dt))


@with_exitstack
def _emit(ctx: ExitStack, tc: tile.TileContext, io: dict):
    nc = tc.nc
    xT_d, wq_d, wk_d, wv_d, bq_d, wfc_d, yT_d = (
        io["xT"], io["wq"], io["wk"], io["wv"], io["bq"], io["wfc"], io["yT"])

    sbW = ctx.enter_context(tc.tile_pool(name="sbW", bufs=1))
    sbP = ctx.enter_context(tc.tile_pool(name="sbP", bufs=1))
    xt_pool = ctx.enter_context(tc.tile_pool(name="xt", bufs=1))
    # separate P^T rings per exp engine (see SCH_PAIRS)
    pt_pool_s = ctx.enter_context(tc.tile_pool(name="pts", bufs=6))
    pt_pool_v = ctx.enter_context(tc.tile_pool(name="ptv", bufs=6))
    ev_pool = ctx.enter_context(tc.tile_pool(name="ev", bufs=6))
    nrm_pool = ctx.enter_context(tc.tile_pool(name="nrm", bufs=2))
    mm_ps = ctx.enter_context(tc.tile_pool(name="mmps", bufs=2, space="PSUM"))
    s_ps = ctx.enter_context(tc.tile_pool(name="sps", bufs=2, space="PSUM"))
    o_ps = ctx.enter_context(tc.tile_pool(name="ops", bufs=2, space="PSUM"))

    # ---- resident x^T (bf16, 4 MB, one [128, 8, 2048] tensor) + weights,
    # pre-cast to bf16 on the host. DMA queue plan (per-engine streams are
    # in-order, so issue order = need order):
    #   sync:   x0, bq, x1, x3   (x0 gates the first Q/K chain)
    #   scalar: wq, wk, wv, x2   (weights first; the x2 descriptor retires
    #                             before the first scalar compute is due)
    #   gpsimd: (wfc later, after pass A)
    xt_all = xt_pool.tile([128, EC, S], DT_X, name="xt", tag="xt")
    xTv = xT_d.rearrange("(ec p) s -> p ec s", p=128)
    wq_all = sbW.tile([128, EC, DG], DT_X, name="wq", tag="wq")
    wk_all = sbW.tile([128, EC, DG], DT_X, name="wk", tag="wk")
    wv_all = sbW.tile([128, EC, DG], DT_X, name="wv", tag="wv")
    def load_w(wt, wd, half, eng=None):
        (eng or nc.scalar).dma_start(
            wt[:, half * 4:(half + 1) * 4, :],
            wd.rearrange("(ec p) d -> p ec d",
                         p=128)[:, half * 4:(half + 1) * 4, :])

    # ---- PE warm-up: the HAM clock gate starts at half clock and only
    # ramps after ~3.4us of sustained matmul activity. Burn the DMA
    # lead-in (PE is idle anyway) on zero matmuls so the gate is warm
    # when real operands land. ----
    wu_sb = sbW.tile([128, 512], DT_X, name="wu", tag="wu")
    nc.vector.memset(wu_sb[:], 0.0)
    for i in range(N_WARMUP):
        wu_ps = o_ps.tile([65, 512], F32, name="wu_ps", tag="po")
        nc.tensor.matmul(wu_ps[:], wu_sb[:, 0:65], wu_sb[:],
                         start=True, stop=True)

    nc.sync.dma_start(xt_all[:, :, 0:512], xTv[:, :, 0:512])
    load_w(wq_all, wq_d, 0)
    load_w(wq_all, wq_d, 1)
    # bq as [128, 4]: column dt holds the 128 bias values of head-pair dt
    # (f32 so it can be the scalar-activation bias on the f32 psum)
    bq_t = sbW.tile([128, DTL], F32, name="bq", tag="bq")
    nc.sync.dma_start(bq_t[:], bq_d.rearrange("o (a p) -> (o p) a", p=128))
    load_w(wk_all, wk_d, 0)
    load_w(wk_all, wk_d, 1)
    load_w(wv_all, wv_d, 0)
    load_w(wv_all, wv_d, 1)
    nc.sync.dma_start(xt_all[:, :, 512:1024], xTv[:, :, 512:1024])
    # x2 rides the gpsimd queue (idle until the first normalize) so the
    # three bulk streams load in parallel under the shared HBM cap
    nc.gpsimd.dma_start(xt_all[:, :, 1024:1536], xTv[:, :, 1024:1536])
    nc.sync.dma_start(xt_all[:, :, 1536:2048], xTv[:, :, 1536:2048])
    xt_res = [[xt_all[:, ec, sc * 512:(sc + 1) * 512] for sc in range(SC)]
              for ec in range(EC)]
    wq_t = [wq_all[:, ec, :] for ec in range(EC)]
    wk_t = [wk_all[:, ec, :] for ec in range(EC)]
    wv_t = [wv_all[:, ec, :] for ec in range(EC)]

    # ---- persistent activations (per-s-chunk tiles so attention's
    # dependencies are fine-grained and can overlap the projection) ----
    QT = [[sbP.tile([128, 512], DT_ATTN, name=f"QT{i}_{s}", tag=f"QT{i}_{s}")
           for s in range(SC)] for i in range(DTL)]
    KT = [[sbP.tile([128, 512], DT_ATTN, name=f"KT{i}_{s}", tag=f"KT{i}_{s}")
           for s in range(SC)] for i in range(DTL)]
    # V_aug in fp8 (values pre-scaled x32 on host; wfc carries 1/32), packed
    # as DoubleRow k-tile pairs: VA2[tp][:, ko, h*66+m] = V_aug for k-tile
    # 2*tp+ko, head h, column m (64 V cols + ones col at m=64; m=65 pad).
    # The ko stride (528) is a multiple of 16 as DoubleRow LDWEIGHTS needs.
    VA2 = [sbP.tile([128, 2, 8 * 66], FP8, name=f"VA2{i}", tag=f"VA2{i}")
           for i in range(ST // 2)]
    OT = [[sbP.tile([128, 512], DT_PROJ, name=f"OT{i}_{s}", tag=f"OT{i}_{s}")
           for s in range(SC)] for i in range(DTL)]

    # ones columns of V_aug (col 64 of each head's 66-col block)
    for tp in range(ST // 2):
        va4 = VA2[tp].rearrange("p ko (h c) -> p ko h c", c=66)
        nc.vector.memset(va4[:, :, :, 64:65], 1.0)

    # ---- projection pieces ----
    def emit_qk(dt_i, sc):
        """Q^T (with bias) and K^T for one head-pair tile, one s-chunk."""
        dsl = slice(dt_i * 128, (dt_i + 1) * 128)
        pq = mm_ps.tile([128, 512], F32, name="pq", tag="mm")
        for ec in range(EC):
            nc.tensor.matmul(pq[:], wq_t[ec][:, dsl], xt_res[ec][sc][:],
                             start=(ec == 0), stop=(ec == EC - 1))
        nc.vector.tensor_scalar_add(QT[dt_i][sc][:], pq[:],
                                    bq_t[:, dt_i:dt_i + 1])
        pk = mm_ps.tile([128, 512], F32, name="pk", tag="mm")
        for ec in range(EC):
            nc.tensor.matmul(pk[:], wk_t[ec][:, dsl], xt_res[ec][sc][:],
                             start=(ec == 0), stop=(ec == EC - 1))
        nc.vector.tensor_copy(KT[dt_i][sc][:], pk[:])

    def emit_v(sc):
        for st_l in range(4):
            st = sc * 4 + st_l
            ssl = slice(st_l * 128, (st_l + 1) * 128)
            pv = mm_ps.tile([128, 512], F32, name="pv", tag="mm")
            for ec in range(EC):
                nc.tensor.matmul(pv[:], xt_res[ec][sc][:, ssl], wv_t[ec][:],
                                 start=(ec == 0), stop=(ec == EC - 1))
            va4 = VA2[st // 2].rearrange("p ko (h c) -> p ko h c", c=66)
            pv3 = pv.rearrange("p (h d) -> p h d", d=64)
            nc.vector.tensor_copy(va4[:, st % 2, :, 0:64], pv3[:])

    def emit_fc(sc, tail=False):
        s0 = sc * 512
        for nt in range(NT):
            nsl = slice(nt * 128, (nt + 1) * 128)
            py = mm_ps.tile([128, 512], F32, name="py", tag="mm")
            for dt_i in range(DTL):
                nc.tensor.matmul(py[:], wfc_t[dt_i][:, nsl],
                                 OT[dt_i][sc][:],
                                 start=(dt_i == 0), stop=(dt_i == DTL - 1))
            yv = ev_pool.tile([128, 512], F32, name="yv", tag="yv")
            # in the tail both exp engines are drained: split the psum
            # evacuation so the mm-psum ring recycles twice as fast
            if tail and nt % 2 == 0:
                nc.scalar.copy(yv[:], py[:])
            else:
                nc.vector.tensor_copy(yv[:], py[:])
            nc.sync.dma_start(yT_d[nt * 128:(nt + 1) * 128, s0:s0 + 512],
                              yv[:])

    def attn_group(hp, qc):
        """Attention for one (head-pair, q-chunk) group, split-emittable.

        Returns (pairs, finish): pairs(lo, hi) emits S/exp/PV for k-tile
        pairs [lo, hi); finish() flushes the PV pipeline and normalizes.
        Callers must emit pairs in order and only after the KT/VA2 tiles
        for that k-range have been emitted (program-order dependency).
        """
        po = [o_ps.tile([65, 512], F32, name=f"po{p}", tag="po")
              for p in range(2)]

        def emit_pv(tp, pt2):
            # DoubleRow PV: one fp8 matmul per head covers k-tile pair
            # tp (256 keys). Stationary = VA2 pair slice [128, 2, 65];
            # moving = P^T pair [128, 2, 512] (ko-strided AP).
            pv4 = pt2.rearrange("p (ko h q) -> p h ko q", ko=2, h=2)
            va4 = VA2[tp].rearrange("p ko (h c) -> p ko h c", c=66)
            for p in range(2):
                h_l = hp * 2 + p
                nc.tensor.matmul(po[p][:],
                                 va4[:, :, h_l, 0:65],
                                 pv4[:, p],
                                 start=(tp == 0),
                                 stop=(tp == ST // 2 - 1),
                                 perf_mode=DR)

        # software-pipelined k-tile pairs: S/exp of pair tp are emitted
        # ahead of the PV of pair tp-3/-4; PVs are flushed two pairs at
        # a time so the 64<->128 tiling-mode switches halve.
        pend = []  # [(tp, pt2)] awaiting PV

        def pairs(lo, hi):
            for tp in range(lo, hi):
                # pt2 free-dim layout: ko*1024 + h*512 + q -> the exp
                # engines write contiguous [128, 1024] tiles per k-tile.
                sch = tp in SCH_PAIRS
                pool = pt_pool_v if sch else pt_pool_s
                pt2 = pool.tile([128, 2048], FP8, name="ptt", tag="ptt")
                pt2v = pt2.rearrange("p (ko hq) -> p ko hq", ko=2)
                pt2u = pt2.bitcast(U8).rearrange("p (ko hq) -> p ko hq",
                                                 ko=2)
                for ko in range(2):
                    kt = 2 * tp + ko
                    ps_t = s_ps.tile([128, 1024], F32, name="ps", tag="ps")
                    for p in range(2):
                        psl = slice(p * 64, (p + 1) * 64)
                        nc.tensor.matmul(ps_t[:, p * 512:(p + 1) * 512],
                                         KT[hp][kt // 4][psl, (kt % 4) * 128:
                                                         (kt % 4) * 128 + 128],
                                         QT[hp][qc][psl, :],
                                         start=True, stop=True)
                    if sch:
                        nc.vector.tensor_scalar(pt2u[:, ko], ps_t[:],
                                                SCH_A, SCH_B,
                                                mybir.AluOpType.mult,
                                                mybir.AluOpType.add)
                    else:
                        nc.scalar.activation(pt2v[:, ko], ps_t[:],
                                             mybir.ActivationFunctionType.Exp)
                pend.append((tp, pt2))
                if len(pend) >= 4:
                    emit_pv(*pend.pop(0))
                    emit_pv(*pend.pop(0))

        def finish(tail=False):
            while pend:
                emit_pv(*pend.pop(0))
            _normalize(tail=tail)

        def _normalize(tail=False):
            # Evacuate both PV psums (releases the po banks), batch the
            # two heads' Z rows into one reciprocal, broadcast + multiply
            # on GpSimd. In the tail (nothing left to overlap) keep the
            # chain on the drained Scalar/Vector engines for min latency.
            posb = [nrm_pool.tile([65, 512], F32, name=f"posb{p}",
                                  tag=f"posb{p}") for p in range(2)]
            if tail:
                nc.scalar.copy(posb[0][:], po[0][:])
                nc.vector.tensor_copy(posb[1][:], po[1][:])
            else:
                for p in range(2):
                    nc.vector.tensor_copy(posb[p][:], po[p][:])
            # custom DVE ops and partition_broadcast read the tensor's
            # partition 0 regardless of AP offset -> move Z via DMA first
            rz2 = nrm_pool.tile([1, 1024], F32, name="rz2", tag="rz2")
            for p in range(2):
                nc.sync.dma_start(rz2[:, p * 512:(p + 1) * 512],
                                  posb[p][64:65, :])
            rzr = nrm_pool.tile([1, 1024], F32, name="rzr", tag="rzr")
            nc.vector.reciprocal_approx_fast(rzr[:], rz2[:])
            rzb = [nrm_pool.tile([64, 512], F32, name=f"rzb{p}",
                                 tag=f"rzb{p}") for p in range(2)]
            beng = nc.gpsimd
            meng = nc.vector if tail else nc.gpsimd
            for p in range(2):
                beng.partition_broadcast(
                    rzb[p][:], rzr[0:1, p * 512:(p + 1) * 512])
            tmp = nrm_pool.tile([64, 512], DT_PROJ, name="otmp", tag="otmp")
            meng.tensor_mul(OT[hp][qc][0:64, :], posb[0][0:64, :],
                            rzb[0][:])
            meng.tensor_mul(tmp[:], posb[1][0:64, :], rzb[1][:])
            # engines cannot shift partitions; DMA moves rows 0:64 into
            # OT rows 64:128.
            nc.sync.dma_start(OT[hp][qc][64:128, :], tmp[:])

        return pairs, finish

    def emit_attn(hp, qc):
        pairs, finish = attn_group(hp, qc)
        pairs(0, ST // 2)
        finish()

    # ---- pass A: V (all heads) + Q/K for head-pair 0, with head-pair
    # 0's first attention group interleaved so the exp engines start as
    # soon as the first chunk's Q/K land (S/exp of pairs [lo,hi) only
    # needs KT; the PV of pair tp is emitted a few pairs later, by which
    # point the V chains covering it are in program order). ----
    g0_pairs, g0_finish = attn_group(0, 0)
    for sc in range(SC):
        emit_qk(0, sc)
        emit_v(sc)
        g0_pairs(2 * sc, 2 * sc + 2)
    g0_finish()
    for qc in range(1, SC):
        emit_attn(0, qc)

    # wfc loads deferred past pass A: first FC use is in the hp3 window,
    # and these 2MB would otherwise crowd the DMA-paced startup. On the
    # sync queue they naturally sit behind the x chunks + g0's moves.
    wfc_t = []
    for dt_i in range(DTL):
        t = sbW.tile([128, E], DT_PROJ, name=f"wfc{dt_i}", tag=f"wfc{dt_i}")
        nc.sync.dma_start(t[:], wfc_d[dt_i * 128:(dt_i + 1) * 128, :])
        wfc_t.append(t)

    # ---- attention interleaved with deferred projections ----
    # Attention for head-pair hp runs while the projection for head-pair
    # hp+1 (emitted just after, lower priority) fills PE gaps. In the
    # last head-pair, the FC of the previous q-chunk is emitted into the
    # middle of the group so it overlaps the exp-bound stretch instead of
    # piling up after the last group.
    for hp in range(1, DTL):
        for sc in range(SC):
            emit_qk(hp, sc)
        for qc in range(SC):
            last = (hp == DTL - 1 and qc == SC - 1)
            pairs, finishg = attn_group(hp, qc)
            pairs(0, 4)
            if hp == DTL - 1 and qc >= 1:
                emit_fc(qc - 1)
            pairs(4, ST // 2)
            finishg(tail=last)
    emit_fc(SC - 1, tail=True)

_CACHE = {}


def _build():
    if "nc" in _CACHE:
        return _CACHE["nc"]
    nc = bacc.Bacc("TRN2", target_bir_lowering=False, debug=False)
    io = {
        "xT": nc.dram_tensor("xT", [E, S], BF16, kind="ExternalInput").ap(),
        "wq": nc.dram_tensor("wq", [E, DG], BF16, kind="ExternalInput").ap(),
        "wk": nc.dram_tensor("wk", [E, DG], BF16, kind="ExternalInput").ap(),
        "wv": nc.dram_tensor("wv", [E, DG], BF16, kind="ExternalInput").ap(),
        "bq": nc.dram_tensor("bq", [1, DG], F32, kind="ExternalInput").ap(),
        "wfc": nc.dram_tensor("wfc", [DG, E], BF16,
                              kind="ExternalInput").ap(),
        "yT": nc.dram_tensor("yT", [E, S], F32, kind="ExternalOutput").ap(),
    }
    with tile.TileContext(nc) as tc:
        _emit(tc, io)
    nc.compile()
    _CACHE["nc"] = nc
    return nc


def make_in_maps(x, w_qkv, b_qkv, w_fc):
    """Host-side sharding: returns per-core input dicts (core = b*G + g)."""
    import ml_dtypes
    x = np.asarray(x, dtype=np.float32)
    w_qkv = np.asarray(w_qkv, dtype=np.float32)
    b_qkv = np.asarray(b_qkv, dtype=np.float32)
    w_fc = np.asarray(w_fc, dtype=np.float32)
    npdt = ml_dtypes.bfloat16
    in_maps = []
    for b in range(B):
        xTb = np.ascontiguousarray(x[b].T).astype(npdt)
        for g in range(G):
            gs = slice(g * DG, (g + 1) * DG)
            in_maps.append({
                "xT": xTb,
                "wq": np.ascontiguousarray(
                    w_qkv[:, 0 * E:1 * E][:, gs] * (1.0 / np.sqrt(D))
                ).astype(npdt),
                "wk": np.ascontiguousarray(w_qkv[:, 1 * E:2 * E][:, gs]).astype(npdt),
                # V path scaled x32 so the fp8(e4m3) V_aug values sit in the
                # normal range; wfc carries the 1/32 back out.
                "wv": np.ascontiguousarray(
                    w_qkv[:, 2 * E:3 * E][:, gs] * 32.0).astype(npdt),
                "bq": np.ascontiguousarray(
                    b_qkv[0 * E:1 * E][gs][None, :] * (1.0 / np.sqrt(D))
                ).astype(np.float32),
                "wfc": np.ascontiguousarray(
                    w_fc[gs, :] * (1.0 / 32.0)).astype(npdt),
            })
    return in_maps


def gather(results, b_qkv, w_fc, b_fc):
    """Host-side unshard: sum group partials, transpose, add const bias."""
    b_qkv = np.asarray(b_qkv, dtype=np.float32)
    w_fc = np.asarray(w_fc, dtype=np.float32)
    b_fc = np.asarray(b_fc, dtype=np.float32)
    cbias = (b_qkv[2 * E:3 * E].astype(np.float64) @ w_fc.astype(np.float64)
             + b_fc.astype(np.float64)).astype(np.float32)
    y = np.empty((B, S, E), np.float32)
    for b in range(B):
        yT = results[b * G]["yT"] + results[b * G + 1]["yT"]
        y[b] = yT.T + cbias[None, :]
    return y


def kernel(x, w_qkv, b_qkv, w_fc, b_fc, _trace=False, _tmpdir=None):
    nc = _build()
    in_maps = make_in_maps(x, w_qkv, b_qkv, w_fc)
    res = run_bass_kernel_spmd(nc, in_maps, list(range(NCORES)),
                               trace=_trace, tmpdir=_tmpdir)
    y = gather(res.results, b_qkv, w_fc, b_fc)
    kernel.last_exec_time_ns = res.exec_time_ns
    kernel.last_res = res
    return y
